# revision 10
# baseline (speedup 1.0000x reference)
"""Causal self-attention (B=2, T=4096, C=512, H=8, Dh=64) on 8 trn2 cores.

Sharding: core = (batch, head-pair). 2 batches x 4 head-pairs = 8 cores.
Each core computes q/k/v projections for its 2 heads, causal attention in
S^T ([k, q]) layout, and a row-parallel slice of the output projection.
Host sums the 4 partial outputs per batch (+ b_out) and stacks batches.

v2 (from trace analysis of v1: ACT exp is the per-core floor ~160us;
PE was inflated by HAM cold-clock + full-rectangle work; DVE by a 3.3us
[1,512] reciprocal per head-tile):
  - attention operands in bf16 (QT/KT/V/expS/YTn/wo), f32 PSUM accum.
    Whole-pipeline bf16 rounding measured 3.9e-3 rel err in numpy.
  - causal restriction: diagonal-chunk S and AV matmuls only stream the
    visible query range [128r, 512); the stale PSUM/e columns are never
    consumed. Mask multiply shrinks to one [128,128] triangle per
    diagonal chunk (one shared bf16 mask; gpsimd).
  - softmax denominator via the appended ones-column in V (row 64 of
    yt PSUM), reciprocal via the single-instruction
    reciprocal_approx_fast instead of the slow exact reciprocal.
  - exp stays on ACT as [128, 2x512] insts reading 2 PSUM banks.
Device pipeline per core otherwise as v1: per query tile qt, chunk-pair
loop S^T -> exp -> (mask) -> yt += V^T-extended @ expS, with Q/K/V/out
projections software-pipelined into the pair slots so PE never idles
long (HAM stays warm).
"""

import os
import sys

import numpy as np
from ml_dtypes import bfloat16

for _p in ("/opt/trn_rl_repo",):
    if os.path.isdir(_p) and _p not in sys.path:
        sys.path.insert(0, _p)

os.environ.setdefault("MYCRO_LOCAL_CACHE", "1")

import concourse.bass as bass  # noqa: E402
from concourse import bacc  # noqa: E402
import concourse.mybir as mybir  # noqa: E402
import concourse.tile as tile  # noqa: E402
from concourse.bass_utils import run_bass_kernel_spmd  # noqa: E402

F32 = mybir.dt.float32
F32R = mybir.dt.float32r
BF16 = mybir.dt.bfloat16

B, T, C, H, DH = 2, 4096, 512, 8, 64
HEADS_PER_CORE = 2
HD = HEADS_PER_CORE * DH  # 128: head dims owned by one core
N_CORES = 8
QT_TILE = 512  # queries per attention tile
KC = 128  # keys per chunk (contraction granularity)
N_QT = T // QT_TILE  # 8
N_KC = T // KC  # 32
CK = C // 128  # 4 contraction chunks for the projections
SCALE = 1.0 / float(np.sqrt(DH))


def build_program():
    nc = bacc.Bacc(None)

    xT = nc.declare_dram_parameter("xT", [C, T], BF16, isOutput=False)
    wqT = nc.declare_dram_parameter("wqT", [C, HD], BF16, isOutput=False)
    wkT = nc.declare_dram_parameter("wkT", [C, HD], BF16, isOutput=False)
    wvT = nc.declare_dram_parameter("wvT", [C, HD], BF16, isOutput=False)
    # woT[d, h, j]: rows of w_out for this core's head dims, head-split so
    # both heads' slices sit on partitions 0-63. bf16 (matmul operand).
    woT = nc.declare_dram_parameter("woT", [DH, 2, C], BF16, isOutput=False)
    bq = nc.declare_dram_parameter("bq", [HD], F32, isOutput=False)
    bk = nc.declare_dram_parameter("bk", [HD], F32, isOutput=False)
    bv = nc.declare_dram_parameter("bv", [HD], F32, isOutput=False)
    out = nc.declare_dram_parameter("out", [T, C], F32, isOutput=True)

    with tile.TileContext(nc) as tc:
        with (
            tc.tile_pool(name="singles", bufs=1) as singles,
            tc.tile_pool(name="xin", bufs=3) as xin,
            tc.tile_pool(name="exps", bufs=4) as exps,
            tc.tile_pool(name="osb", bufs=3) as osb,
            tc.tile_pool(name="norm", bufs=2) as norm,
            tc.tile_pool(name="ps_proj", bufs=2, space="PSUM") as ps_proj,
            tc.tile_pool(name="ps_s", bufs=2, space="PSUM") as ps_s,
            tc.tile_pool(name="ps_yt", bufs=1, space="PSUM") as ps_yt,
        ):
            # ---- resident inputs -------------------------------------
            # order: wq/wk, then the first x tile (emitted in the qt loop),
            # then everything needed later — keeps the first S chain early
            xT_ap = xT.rearrange("(ko p) t -> p ko t", p=128)
            # weights ride the gpsimd SWDGE ring so they overlap the x-tile
            # loads on the sync HWDGE ring at kernel start.
            wqT_sb = singles.tile([128, CK, HD], BF16)
            nc.gpsimd.dma_start(
                wqT_sb, wqT.rearrange("(ko p) m -> p ko m", p=128)
            )
            # first x tile split per contraction chunk so the first q-proj
            # matmul starts after 128KB instead of 512KB of DMA
            xt_first = xin.tile([128, CK, QT_TILE], BF16, tag="xt", name="xt_first")
            for kc in range(CK):
                nc.sync.dma_start(
                    xt_first[:, kc, :],
                    xT_ap[:, kc, bass.ts(0, QT_TILE)],
                )
            wkT_sb = singles.tile([128, CK, HD], BF16)
            nc.gpsimd.dma_start(
                wkT_sb, wkT.rearrange("(ko p) m -> p ko m", p=128)
            )
            wvT_sb = singles.tile([128, CK, HD], BF16)
            nc.gpsimd.dma_start(
                wvT_sb, wvT.rearrange("(ko p) m -> p ko m", p=128)
            )
            woT_sb = singles.tile([DH, 2, C], BF16)
            nc.gpsimd.dma_start(woT_sb, woT[:])

            bq_col = singles.tile([128, 1], F32)
            nc.gpsimd.dma_start(bq_col, bq.rearrange("(p one) -> p one", one=1))
            bk_col = singles.tile([128, 1], F32)
            nc.gpsimd.dma_start(bk_col, bk.rearrange("(p one) -> p one", one=1))
            bv_row = singles.tile([1, HD], F32R)
            nc.gpsimd.dma_start(bv_row, bv[None, :].bitcast(F32R))

            ones_f32 = singles.tile([128, 128], F32)
            nc.vector.memset(ones_f32, 1.0)
            ones_row = singles.tile([128, 128], F32R)
            nc.vector.tensor_copy(ones_row, ones_f32)

            # one shared [128,128] lower-triangle 0/1 mask (bf16): within a
            # diagonal chunk r the visible block is always q_local >= k_local.
            # Built in F32 (affine_select limitation), then cast.
            tri_f32 = singles.tile([128, 128], F32)
            nc.vector.memset(tri_f32, 1.0)
            nc.gpsimd.affine_select(
                out=tri_f32,
                in_=tri_f32,
                compare_op=mybir.AluOpType.is_ge,
                fill=0.0,
                base=0,
                pattern=[[1, 128]],
                channel_multiplier=-1,
            )
            tri_bf = singles.tile([128, 128], BF16)
            nc.vector.tensor_copy(tri_bf, tri_f32)

            # broadcast bv across partitions via a K=1 matmul
            bias_v_ps = ps_proj.tile([128, HD], F32, tag="psproj")
            nc.tensor.matmul(
                bias_v_ps, ones_row[0:1, :], bv_row, start=True, stop=True
            )
            bias_v_sb = singles.tile([128, HD], F32)
            nc.vector.tensor_copy(bias_v_sb, bias_v_ps)
            bias_v2 = bias_v_sb.rearrange("p (h x) -> p h x", h=2)

            # per-tile storage (separate tile objects -> precise deps)
            QT_t = [
                singles.tile([128, QT_TILE], BF16, name=f"qtt{i}", tag=f"qtt{i}")
                for i in range(N_QT)
            ]
            KT_t = [
                singles.tile([128, QT_TILE], BF16, name=f"ktt{i}", tag=f"ktt{i}")
                for i in range(N_QT)
            ]
            # V chunks in [k, d] layout; per tile: 4 chunks of
            # [V0 | ones | V1 | ones] (65-column stride per head slice)
            V_t = [
                singles.tile([128, 4, 130], BF16, name=f"vt{i}", tag=f"vt{i}")
                for i in range(N_QT)
            ]
            YTn_t = [
                [
                    singles.tile(
                        [64, QT_TILE], BF16, name=f"ytn{h}_{i}", tag=f"ytn{h}_{i}"
                    )
                    for i in range(N_QT)
                ]
                for h in range(2)
            ]
            for i in range(N_QT):
                nc.vector.tensor_copy(V_t[i][:, :, 64:65], ones_f32[:, 0:4, None])
                nc.vector.tensor_copy(
                    V_t[i][:, :, 129:130], ones_f32[:, 0:4, None]
                )

            def emit_qproj(qt, xt):
                ps_q = ps_proj.tile([128, QT_TILE], F32, tag="psproj", name="ps_q")
                for kc in range(CK):
                    nc.tensor.matmul(
                        ps_q,
                        wqT_sb[:, kc, :],
                        xt[:, kc, :],
                        start=(kc == 0),
                        stop=(kc == CK - 1),
                    )
                nc.vector.tensor_scalar_add(QT_t[qt][:], ps_q, bq_col)

            def emit_kproj(qt, xt):
                ps_k = ps_proj.tile([128, QT_TILE], F32, tag="psproj", name="ps_k")
                for kc in range(CK):
                    nc.tensor.matmul(
                        ps_k,
                        wkT_sb[:, kc, :],
                        xt[:, kc, :],
                        start=(kc == 0),
                        stop=(kc == CK - 1),
                    )
                nc.vector.tensor_scalar_add(KT_t[qt][:], ps_k, bk_col)

            def emit_vproj(qt, xt, sv):
                ps_v = ps_proj.tile([128, HD], F32, tag="psproj", name="ps_v")
                for kc in range(CK):
                    nc.tensor.matmul(
                        ps_v,
                        xt[:, kc, bass.ts(sv, 128)],
                        wvT_sb[:, kc, :],
                        start=(kc == 0),
                        stop=(kc == CK - 1),
                    )
                vt = V_t[qt]
                v_vals = bass.AP(
                    tensor=vt.tensor,
                    offset=vt.offset,
                    ap=[vt.ap[0], vt.ap[1], [65, 2], [1, 64]],
                )
                nc.vector.tensor_add(
                    v_vals[:, sv],
                    ps_v[:, 0:HD].rearrange("p (h x) -> p h x", h=2),
                    bias_v2,
                )

            def emit_outproj_sv(qt, sv):
                tc8 = qt * (QT_TILE // 128) + sv
                ps_o = ps_proj.tile([128, C], F32, tag="psproj", name="ps_o")
                for h in range(2):
                    nc.tensor.matmul(
                        ps_o,
                        YTn_t[h][qt][:, bass.ts(sv, 128)],
                        woT_sb[:, h, :],
                        start=(h == 0),
                        stop=(h == 1),
                    )
                o_sb = osb.tile([128, C], F32, tag="osb")
                nc.vector.tensor_copy(o_sb, ps_o)
                nc.sync.dma_start(out[bass.ts(tc8, 128), :], o_sb)

            xt_tiles = {0: xt_first}

            def emit_xt(i):
                if i not in xt_tiles and i < N_QT:
                    xt_i = xin.tile(
                        [128, CK, QT_TILE], BF16, tag="xt", name=f"xt{i}"
                    )
                    nc.sync.dma_start(
                        xt_i, xT_ap[:, :, bass.ts(i, QT_TILE)]
                    )
                    xt_tiles[i] = xt_i

            qproj_done = set()
            for qt in range(N_QT):
                emit_xt(qt)
                xt = xt_tiles[qt]
                if qt not in qproj_done:
                    emit_qproj(qt, xt)
                    qproj_done.add(qt)
                if qt == 0:
                    emit_kproj(qt, xt)
                    for sv in range(4):
                        emit_vproj(qt, xt, sv)

                yt_ps = [
                    ps_yt.tile([128, QT_TILE], F32, tag=f"yt{h}", name=f"yt{h}")
                    for h in range(2)
                ]
                n_pairs = 2 * (qt + 1)
                # spread the pipelined projection work across pair slots so
                # tile boundaries don't burst 30+ matmuls into one slot.
                # deadlines: V chunk sv is consumed at pair 2qt + sv//2;
                # K^T(qt) at pair 2qt (first diagonal pair).
                vproj_at = {}
                outproj_at = {}
                if qt > 0:
                    for sv in range(4):
                        p = min(1 + sv // 2, 2 * qt + sv // 2)
                        vproj_at.setdefault(p, []).append(sv)
                    base = 1 if qt == 1 else 3
                    for sv in range(4):
                        outproj_at.setdefault(
                            min(base + sv, n_pairs - 1), []
                        ).append(sv)
                for pair in range(n_pairs):
                    s_ps = [
                        ps_s.tile(
                            [128, 2, QT_TILE], F32, tag="s", name=f"s{h}"
                        )
                        for h in range(2)
                    ]
                    # sub-major order: the two heads' S matmuls sit on
                    # disjoint PE row-groups (partitions 0-63 / 64-127), so
                    # adjacent emission lets them execute concurrently.
                    ctx_pri = tc.high_priority(offset=8)
                    ctx_pri.__enter__()
                    for sub in range(2):
                        c = pair * 2 + sub
                        r = c - 4 * qt
                        off = 128 * r if r > 0 else 0
                        for h in range(2):
                            hp = slice(h * 64, h * 64 + 64)
                            nc.tensor.matmul(
                                s_ps[h][:, sub, off:QT_TILE],
                                KT_t[c // 4][hp, bass.ts(c % 4, KC)],
                                QT_t[qt][hp, off:QT_TILE],
                                start=True,
                                stop=True,
                            )
                    ctx_pri.__exit__(None, None, None)
                    # pipelined projections / out-proj for other tiles
                    if pair == 0 and qt > 0:
                        emit_kproj(qt, xt)
                    e_sb = [
                        exps.tile(
                            [128, 2, QT_TILE], BF16, tag=f"e{h}", name=f"e{h}"
                        )
                        for h in range(2)
                    ]
                    for h in range(2):
                        nc.scalar.activation(
                            e_sb[h],
                            s_ps[h],
                            mybir.ActivationFunctionType.Exp,
                            scale=SCALE,
                        )
                        for sub in range(2):
                            c = pair * 2 + sub
                            r = c - 4 * qt
                            if r >= 0:  # diagonal chunk: zero the 128-wide
                                # triangle block (cols < 128r are never read)
                                blk = slice(128 * r, 128 * r + 128)
                                nc.gpsimd.tensor_mul(
                                    e_sb[h][:, sub, blk],
                                    e_sb[h][:, sub, blk],
                                    tri_bf,
                                )
                    for sv in vproj_at.get(pair, []):
                        emit_vproj(qt, xt, sv)
                    for h in range(2):
                        for sub in range(2):
                            c = pair * 2 + sub
                            r = c - 4 * qt
                            off = 128 * r if r > 0 else 0
                            nc.tensor.matmul(
                                yt_ps[h][0:65, off:QT_TILE],
                                V_t[c // 4][:, c % 4, h * 65 : h * 65 + 65],
                                e_sb[h][:, sub, off:QT_TILE],
                                start=(pair == 0 and sub == 0),
                                stop=(pair == n_pairs - 1 and sub == 1),
                            )
                    for sv in outproj_at.get(pair, []):
                        emit_outproj_sv(qt - 1, sv)
                    if pair == min(2, n_pairs - 1) and qt + 1 < N_QT:
                        emit_xt(qt + 1)
                        emit_qproj(qt + 1, xt_tiles[qt + 1])
                        qproj_done.add(qt + 1)

                # ---- normalize (row 64 = softmax denominator) --------
                # den -> SBUF (f32r for the broadcast matmul), PE-broadcast
                # to 64 partitions, then one approx-reciprocal on the
                # broadcast block and the scaling multiply.
                den_sb = norm.tile([128, 2, QT_TILE], F32R, tag="recip")
                bc_sb = [
                    norm.tile([64, QT_TILE], F32, tag=f"bc{h}", name=f"bc{h}")
                    for h in range(2)
                ]
                for h in range(2):
                    nc.vector.tensor_copy(
                        den_sb[64:65, h, :], yt_ps[h][64:65, :]
                    )
                    bc_ps = ps_proj.tile(
                        [64, QT_TILE], F32, tag="psproj", name="bc_ps"
                    )
                    nc.tensor.matmul(
                        bc_ps,
                        ones_row[64:65, 0:64],
                        den_sb[64:65, h, :],
                        start=True,
                        stop=True,
                    )
                    nc.vector.reciprocal_approx_fast(bc_sb[h], bc_ps)
                    nc.vector.tensor_mul(
                        YTn_t[h][qt][:], yt_ps[h][0:64, :], bc_sb[h]
                    )
            for sv in range(4):
                emit_outproj_sv(N_QT - 1, sv)

    return nc


_PROGRAM = None


def _get_program():
    global _PROGRAM
    if _PROGRAM is None:
        _PROGRAM = build_program()
        if not _PROGRAM.is_finalized():
            _PROGRAM.finalize()
    return _PROGRAM


def make_in_maps(x, w_qkv, b_qkv, w_out, b_out):
    """Shard the full inputs into per-core input maps."""
    x = np.ascontiguousarray(x, dtype=np.float32)
    w_qkv = np.ascontiguousarray(w_qkv, dtype=np.float32)
    b_qkv = np.ascontiguousarray(b_qkv, dtype=np.float32)
    w_out = np.ascontiguousarray(w_out, dtype=np.float32)

    wq = w_qkv[0:C]  # [C, C] rows = q features
    wk = w_qkv[C : 2 * C]
    wv = w_qkv[2 * C : 3 * C]
    bq_full = b_qkv[0:C]
    bk_full = b_qkv[C : 2 * C]
    bv_full = b_qkv[2 * C : 3 * C]

    xT_b = [np.ascontiguousarray(x[b].T.astype(bfloat16)) for b in range(B)]

    in_maps = []
    for core in range(N_CORES):
        b = core // 4
        g = core % 4
        rows = slice(g * HD, (g + 1) * HD)  # this core's head dims
        woT = np.ascontiguousarray(
            w_out[:, rows].T.reshape(2, DH, C).transpose(1, 0, 2)
        ).astype(bfloat16)  # [DH, 2, C]
        in_maps.append(
            {
                "xT": xT_b[b],
                "wqT": np.ascontiguousarray(wq[rows].T.astype(bfloat16)),
                "wkT": np.ascontiguousarray(wk[rows].T.astype(bfloat16)),
                "wvT": np.ascontiguousarray(wv[rows].T.astype(bfloat16)),
                "woT": woT,
                "bq": np.ascontiguousarray(bq_full[rows]),
                "bk": np.ascontiguousarray(bk_full[rows]),
                "bv": np.ascontiguousarray(bv_full[rows]),
            }
        )
    return in_maps


def kernel(x, w_qkv, b_qkv, w_out, b_out, _trace=False, _trace_kwargs=None):
    in_maps = make_in_maps(x, w_qkv, b_qkv, w_out, b_out)
    nc = _get_program()
    res = run_bass_kernel_spmd(
        nc,
        in_maps,
        list(range(N_CORES)),
        trace=_trace,
        **(_trace_kwargs or {}),
    )
    outs = [res.results[c]["out"] for c in range(N_CORES)]
    bo = np.asarray(b_out, dtype=np.float32)
    # unshard: sum the 4 row-parallel partials per batch (+ bias), stack
    y = np.stack(
        [
            outs[0] + outs[1] + outs[2] + outs[3] + bo,
            outs[4] + outs[5] + outs[6] + outs[7] + bo,
        ]
    ).astype(np.float32)
    if _trace:
        return y, res
    return y


# revision 11
# speedup vs baseline: 1.0263x; 1.0263x over previous
"""Causal self-attention (B=2, T=4096, C=512, H=8, Dh=64) on 8 trn2 cores.

Sharding: core = (batch, head-pair). 2 batches x 4 head-pairs = 8 cores.
Each core computes q/k/v projections for its 2 heads, causal attention in
S^T ([k, q]) layout, and a row-parallel slice of the output projection.
Host sums the 4 partial outputs per batch (+ b_out) and stacks batches.

v2 (from trace analysis of v1: ACT exp is the per-core floor ~160us;
PE was inflated by HAM cold-clock + full-rectangle work; DVE by a 3.3us
[1,512] reciprocal per head-tile):
  - attention operands in bf16 (QT/KT/V/expS/YTn/wo), f32 PSUM accum.
    Whole-pipeline bf16 rounding measured 3.9e-3 rel err in numpy.
  - causal restriction: diagonal-chunk S and AV matmuls only stream the
    visible query range [128r, 512); the stale PSUM/e columns are never
    consumed. Mask multiply shrinks to one [128,128] triangle per
    diagonal chunk (one shared bf16 mask; gpsimd).
  - softmax denominator via the appended ones-column in V (row 64 of
    yt PSUM), reciprocal via the single-instruction
    reciprocal_approx_fast instead of the slow exact reciprocal.
  - exp stays on ACT as [128, 2x512] insts reading 2 PSUM banks.
Device pipeline per core otherwise as v1: per query tile qt, chunk-pair
loop S^T -> exp -> (mask) -> yt += V^T-extended @ expS, with Q/K/V/out
projections software-pipelined into the pair slots so PE never idles
long (HAM stays warm).
"""

import os
import sys

import numpy as np
from ml_dtypes import bfloat16

for _p in ("/opt/trn_rl_repo",):
    if os.path.isdir(_p) and _p not in sys.path:
        sys.path.insert(0, _p)

os.environ.setdefault("MYCRO_LOCAL_CACHE", "1")

import concourse.bass as bass  # noqa: E402
from concourse import bacc  # noqa: E402
import concourse.mybir as mybir  # noqa: E402
import concourse.tile as tile  # noqa: E402
from concourse.bass_utils import run_bass_kernel_spmd  # noqa: E402

F32 = mybir.dt.float32
F32R = mybir.dt.float32r
BF16 = mybir.dt.bfloat16

B, T, C, H, DH = 2, 4096, 512, 8, 64
HEADS_PER_CORE = 2
HD = HEADS_PER_CORE * DH  # 128: head dims owned by one core
N_CORES = 8
QT_TILE = 512  # queries per attention tile
KC = 128  # keys per chunk (contraction granularity)
N_QT = T // QT_TILE  # 8
N_KC = T // KC  # 32
CK = C // 128  # 4 contraction chunks for the projections
SCALE = 1.0 / float(np.sqrt(DH))


def build_program():
    nc = bacc.Bacc(None)

    xT = nc.declare_dram_parameter("xT", [C, T], BF16, isOutput=False)
    wqT = nc.declare_dram_parameter("wqT", [C, HD], BF16, isOutput=False)
    wkT = nc.declare_dram_parameter("wkT", [C, HD], BF16, isOutput=False)
    wvT = nc.declare_dram_parameter("wvT", [C, HD], BF16, isOutput=False)
    # woT[d, h, j]: rows of w_out for this core's head dims, head-split so
    # both heads' slices sit on partitions 0-63. bf16 (matmul operand).
    woT = nc.declare_dram_parameter("woT", [DH, 2, C], BF16, isOutput=False)
    bq = nc.declare_dram_parameter("bq", [HD], F32, isOutput=False)
    bk = nc.declare_dram_parameter("bk", [HD], F32, isOutput=False)
    bv = nc.declare_dram_parameter("bv", [HD], F32, isOutput=False)
    out = nc.declare_dram_parameter("out", [T, C], F32, isOutput=True)

    with tile.TileContext(nc) as tc:
        with (
            tc.tile_pool(name="singles", bufs=1) as singles,
            tc.tile_pool(name="xin", bufs=3) as xin,
            tc.tile_pool(name="exps", bufs=4) as exps,
            tc.tile_pool(name="osb", bufs=3) as osb,
            tc.tile_pool(name="norm", bufs=2) as norm,
            tc.tile_pool(name="ps_proj", bufs=2, space="PSUM") as ps_proj,
            tc.tile_pool(name="ps_s", bufs=2, space="PSUM") as ps_s,
            tc.tile_pool(name="ps_yt", bufs=1, space="PSUM") as ps_yt,
        ):
            # ---- resident inputs -------------------------------------
            # order: wq/wk, then the first x tile (emitted in the qt loop),
            # then everything needed later — keeps the first S chain early
            xT_ap = xT.rearrange("(ko p) t -> p ko t", p=128)
            # weights ride the gpsimd SWDGE ring so they overlap the x-tile
            # loads on the sync HWDGE ring at kernel start.
            wqT_sb = singles.tile([128, CK, HD], BF16)
            nc.gpsimd.dma_start(
                wqT_sb, wqT.rearrange("(ko p) m -> p ko m", p=128)
            )
            # first x tile split per contraction chunk so the first q-proj
            # matmul starts after 128KB instead of 512KB of DMA
            xt_first = xin.tile([128, CK, QT_TILE], BF16, tag="xt", name="xt_first")
            for kc in range(CK):
                nc.sync.dma_start(
                    xt_first[:, kc, :],
                    xT_ap[:, kc, bass.ts(0, QT_TILE)],
                )
            wkT_sb = singles.tile([128, CK, HD], BF16)
            nc.gpsimd.dma_start(
                wkT_sb, wkT.rearrange("(ko p) m -> p ko m", p=128)
            )
            wvT_sb = singles.tile([128, CK, HD], BF16)
            nc.gpsimd.dma_start(
                wvT_sb, wvT.rearrange("(ko p) m -> p ko m", p=128)
            )
            woT_sb = singles.tile([DH, 2, C], BF16)
            nc.gpsimd.dma_start(woT_sb, woT[:])

            bq_col = singles.tile([128, 1], F32)
            nc.gpsimd.dma_start(bq_col, bq.rearrange("(p one) -> p one", one=1))
            bk_col = singles.tile([128, 1], F32)
            nc.gpsimd.dma_start(bk_col, bk.rearrange("(p one) -> p one", one=1))
            bv_row = singles.tile([1, HD], F32R)
            nc.gpsimd.dma_start(bv_row, bv[None, :].bitcast(F32R))

            ones_f32 = singles.tile([128, 128], F32)
            nc.vector.memset(ones_f32, 1.0)
            ones_row = singles.tile([128, 128], F32R)
            nc.vector.tensor_copy(ones_row, ones_f32)

            # one shared [128,128] lower-triangle 0/1 mask (bf16): within a
            # diagonal chunk r the visible block is always q_local >= k_local.
            # Built in F32 (affine_select limitation), then cast.
            tri_f32 = singles.tile([128, 128], F32)
            nc.vector.memset(tri_f32, 1.0)
            nc.gpsimd.affine_select(
                out=tri_f32,
                in_=tri_f32,
                compare_op=mybir.AluOpType.is_ge,
                fill=0.0,
                base=0,
                pattern=[[1, 128]],
                channel_multiplier=-1,
            )
            tri_bf = singles.tile([128, 128], BF16)
            nc.vector.tensor_copy(tri_bf, tri_f32)

            # broadcast bv across partitions via a K=1 matmul
            bias_v_ps = ps_proj.tile([128, HD], F32, tag="psproj")
            nc.tensor.matmul(
                bias_v_ps, ones_row[0:1, :], bv_row, start=True, stop=True
            )
            bias_v_sb = singles.tile([128, HD], F32)
            nc.vector.tensor_copy(bias_v_sb, bias_v_ps)
            bias_v2 = bias_v_sb.rearrange("p (h x) -> p h x", h=2)

            # per-tile storage (separate tile objects -> precise deps)
            QT_t = [
                singles.tile([128, QT_TILE], BF16, name=f"qtt{i}", tag=f"qtt{i}")
                for i in range(N_QT)
            ]
            KT_t = [
                singles.tile([128, QT_TILE], BF16, name=f"ktt{i}", tag=f"ktt{i}")
                for i in range(N_QT)
            ]
            # V chunks in [k, d] layout; per tile: 4 chunks of
            # [V0 | ones | V1 | ones] (65-column stride per head slice)
            V_t = [
                singles.tile([128, 4, 130], BF16, name=f"vt{i}", tag=f"vt{i}")
                for i in range(N_QT)
            ]
            YTn_t = [
                [
                    singles.tile(
                        [64, QT_TILE], BF16, name=f"ytn{h}_{i}", tag=f"ytn{h}_{i}"
                    )
                    for i in range(N_QT)
                ]
                for h in range(2)
            ]
            for i in range(N_QT):
                nc.vector.tensor_copy(V_t[i][:, :, 64:65], ones_f32[:, 0:4, None])
                nc.vector.tensor_copy(
                    V_t[i][:, :, 129:130], ones_f32[:, 0:4, None]
                )

            def emit_qproj(qt, xt):
                ps_q = ps_proj.tile([128, QT_TILE], F32, tag="psproj", name="ps_q")
                for kc in range(CK):
                    nc.tensor.matmul(
                        ps_q,
                        wqT_sb[:, kc, :],
                        xt[:, kc, :],
                        start=(kc == 0),
                        stop=(kc == CK - 1),
                    )
                nc.vector.tensor_scalar_add(QT_t[qt][:], ps_q, bq_col)

            def emit_kproj(qt, xt):
                ps_k = ps_proj.tile([128, QT_TILE], F32, tag="psproj", name="ps_k")
                for kc in range(CK):
                    nc.tensor.matmul(
                        ps_k,
                        wkT_sb[:, kc, :],
                        xt[:, kc, :],
                        start=(kc == 0),
                        stop=(kc == CK - 1),
                    )
                nc.vector.tensor_scalar_add(KT_t[qt][:], ps_k, bk_col)

            def emit_vproj(qt, xt, sv):
                ps_v = ps_proj.tile([128, HD], F32, tag="psproj", name="ps_v")
                for kc in range(CK):
                    nc.tensor.matmul(
                        ps_v,
                        xt[:, kc, bass.ts(sv, 128)],
                        wvT_sb[:, kc, :],
                        start=(kc == 0),
                        stop=(kc == CK - 1),
                    )
                vt = V_t[qt]
                v_vals = bass.AP(
                    tensor=vt.tensor,
                    offset=vt.offset,
                    ap=[vt.ap[0], vt.ap[1], [65, 2], [1, 64]],
                )
                nc.vector.tensor_add(
                    v_vals[:, sv],
                    ps_v[:, 0:HD].rearrange("p (h x) -> p h x", h=2),
                    bias_v2,
                )

            def emit_outproj_sv(qt, sv):
                tc8 = qt * (QT_TILE // 128) + sv
                ps_o = ps_proj.tile([128, C], F32, tag="psproj", name="ps_o")
                for h in range(2):
                    nc.tensor.matmul(
                        ps_o,
                        YTn_t[h][qt][:, bass.ts(sv, 128)],
                        woT_sb[:, h, :],
                        start=(h == 0),
                        stop=(h == 1),
                    )
                o_sb = osb.tile([128, C], F32, tag="osb")
                nc.vector.tensor_copy(o_sb, ps_o)
                nc.sync.dma_start(out[bass.ts(tc8, 128), :], o_sb)

            xt_tiles = {0: xt_first}

            def emit_xt(i):
                if i not in xt_tiles and i < N_QT:
                    xt_i = xin.tile(
                        [128, CK, QT_TILE], BF16, tag="xt", name=f"xt{i}"
                    )
                    nc.sync.dma_start(
                        xt_i, xT_ap[:, :, bass.ts(i, QT_TILE)]
                    )
                    xt_tiles[i] = xt_i

            qproj_done = set()
            for qt in range(N_QT):
                emit_xt(qt)
                xt = xt_tiles[qt]
                if qt not in qproj_done:
                    emit_qproj(qt, xt)
                    qproj_done.add(qt)
                if qt == 0:
                    emit_kproj(qt, xt)
                    for sv in range(4):
                        emit_vproj(qt, xt, sv)

                yt_ps = [
                    ps_yt.tile([128, QT_TILE], F32, tag=f"yt{h}", name=f"yt{h}")
                    for h in range(2)
                ]
                n_pairs = 2 * (qt + 1)
                # spread the pipelined projection work across pair slots so
                # tile boundaries don't burst 30+ matmuls into one slot.
                # deadlines: V chunk sv is consumed at pair 2qt + sv//2;
                # K^T(qt) at pair 2qt (first diagonal pair).
                vproj_at = {}
                outproj_at = {}
                if qt > 0:
                    for sv in range(4):
                        p = min(1 + sv // 2, 2 * qt + sv // 2)
                        vproj_at.setdefault(p, []).append(sv)
                    base = 1 if qt == 1 else 3
                    for sv in range(4):
                        outproj_at.setdefault(
                            min(base + sv, n_pairs - 1), []
                        ).append(sv)
                for pair in range(n_pairs):
                    s_ps = [
                        ps_s.tile(
                            [128, 2, QT_TILE], F32, tag="s", name=f"s{h}"
                        )
                        for h in range(2)
                    ]
                    # sub-major order: the two heads' S matmuls sit on
                    # disjoint PE row-groups (partitions 0-63 / 64-127), so
                    # adjacent emission lets them execute concurrently.
                    for sub in range(2):
                        c = pair * 2 + sub
                        r = c - 4 * qt
                        off = 128 * r if r > 0 else 0
                        for h in range(2):
                            hp = slice(h * 64, h * 64 + 64)
                            nc.tensor.matmul(
                                s_ps[h][:, sub, off:QT_TILE],
                                KT_t[c // 4][hp, bass.ts(c % 4, KC)],
                                QT_t[qt][hp, off:QT_TILE],
                                start=True,
                                stop=True,
                            )
                    # pipelined projections / out-proj for other tiles
                    if pair == 0 and qt > 0:
                        emit_kproj(qt, xt)
                    e_sb = [
                        exps.tile(
                            [128, 2, QT_TILE], BF16, tag=f"e{h}", name=f"e{h}"
                        )
                        for h in range(2)
                    ]
                    for h in range(2):
                        nc.scalar.activation(
                            e_sb[h],
                            s_ps[h],
                            mybir.ActivationFunctionType.Exp,
                            scale=SCALE,
                        )
                        for sub in range(2):
                            c = pair * 2 + sub
                            r = c - 4 * qt
                            if r >= 0:  # diagonal chunk: zero the 128-wide
                                # triangle block (cols < 128r are never read)
                                blk = slice(128 * r, 128 * r + 128)
                                nc.gpsimd.tensor_mul(
                                    e_sb[h][:, sub, blk],
                                    e_sb[h][:, sub, blk],
                                    tri_bf,
                                )
                    for sv in vproj_at.get(pair, []):
                        emit_vproj(qt, xt, sv)
                    for h in range(2):
                        for sub in range(2):
                            c = pair * 2 + sub
                            r = c - 4 * qt
                            off = 128 * r if r > 0 else 0
                            nc.tensor.matmul(
                                yt_ps[h][0:65, off:QT_TILE],
                                V_t[c // 4][:, c % 4, h * 65 : h * 65 + 65],
                                e_sb[h][:, sub, off:QT_TILE],
                                start=(pair == 0 and sub == 0),
                                stop=(pair == n_pairs - 1 and sub == 1),
                            )
                    for sv in outproj_at.get(pair, []):
                        emit_outproj_sv(qt - 1, sv)
                    if pair == min(2, n_pairs - 1) and qt + 1 < N_QT:
                        emit_xt(qt + 1)
                        emit_qproj(qt + 1, xt_tiles[qt + 1])
                        qproj_done.add(qt + 1)

                # ---- normalize (row 64 = softmax denominator) --------
                # den -> SBUF (f32r for the broadcast matmul), PE-broadcast
                # to 64 partitions, then one approx-reciprocal on the
                # broadcast block and the scaling multiply.
                den_sb = norm.tile([128, 2, QT_TILE], F32R, tag="recip")
                bc_sb = [
                    norm.tile([64, QT_TILE], F32, tag=f"bc{h}", name=f"bc{h}")
                    for h in range(2)
                ]
                for h in range(2):
                    nc.vector.tensor_copy(
                        den_sb[64:65, h, :], yt_ps[h][64:65, :]
                    )
                    bc_ps = ps_proj.tile(
                        [64, QT_TILE], F32, tag="psproj", name="bc_ps"
                    )
                    nc.tensor.matmul(
                        bc_ps,
                        ones_row[64:65, 0:64],
                        den_sb[64:65, h, :],
                        start=True,
                        stop=True,
                    )
                    nc.vector.reciprocal_approx_fast(bc_sb[h], bc_ps)
                    nc.vector.tensor_mul(
                        YTn_t[h][qt][:], yt_ps[h][0:64, :], bc_sb[h]
                    )
            for sv in range(4):
                emit_outproj_sv(N_QT - 1, sv)

    return nc


_PROGRAM = None


def _get_program():
    global _PROGRAM
    if _PROGRAM is None:
        _PROGRAM = build_program()
        if not _PROGRAM.is_finalized():
            _PROGRAM.finalize()
    return _PROGRAM


def make_in_maps(x, w_qkv, b_qkv, w_out, b_out):
    """Shard the full inputs into per-core input maps."""
    x = np.ascontiguousarray(x, dtype=np.float32)
    w_qkv = np.ascontiguousarray(w_qkv, dtype=np.float32)
    b_qkv = np.ascontiguousarray(b_qkv, dtype=np.float32)
    w_out = np.ascontiguousarray(w_out, dtype=np.float32)

    wq = w_qkv[0:C]  # [C, C] rows = q features
    wk = w_qkv[C : 2 * C]
    wv = w_qkv[2 * C : 3 * C]
    bq_full = b_qkv[0:C]
    bk_full = b_qkv[C : 2 * C]
    bv_full = b_qkv[2 * C : 3 * C]

    xT_b = [np.ascontiguousarray(x[b].T.astype(bfloat16)) for b in range(B)]

    in_maps = []
    for core in range(N_CORES):
        b = core // 4
        g = core % 4
        rows = slice(g * HD, (g + 1) * HD)  # this core's head dims
        woT = np.ascontiguousarray(
            w_out[:, rows].T.reshape(2, DH, C).transpose(1, 0, 2)
        ).astype(bfloat16)  # [DH, 2, C]
        in_maps.append(
            {
                "xT": xT_b[b],
                "wqT": np.ascontiguousarray(wq[rows].T.astype(bfloat16)),
                "wkT": np.ascontiguousarray(wk[rows].T.astype(bfloat16)),
                "wvT": np.ascontiguousarray(wv[rows].T.astype(bfloat16)),
                "woT": woT,
                "bq": np.ascontiguousarray(bq_full[rows]),
                "bk": np.ascontiguousarray(bk_full[rows]),
                "bv": np.ascontiguousarray(bv_full[rows]),
            }
        )
    return in_maps


def kernel(x, w_qkv, b_qkv, w_out, b_out, _trace=False, _trace_kwargs=None):
    in_maps = make_in_maps(x, w_qkv, b_qkv, w_out, b_out)
    nc = _get_program()
    res = run_bass_kernel_spmd(
        nc,
        in_maps,
        list(range(N_CORES)),
        trace=_trace,
        **(_trace_kwargs or {}),
    )
    outs = [res.results[c]["out"] for c in range(N_CORES)]
    bo = np.asarray(b_out, dtype=np.float32)
    # unshard: sum the 4 row-parallel partials per batch (+ bias), stack
    y = np.stack(
        [
            outs[0] + outs[1] + outs[2] + outs[3] + bo,
            outs[4] + outs[5] + outs[6] + outs[7] + bo,
        ]
    ).astype(np.float32)
    if _trace:
        return y, res
    return y


# revision 12
# speedup vs baseline: 1.0624x; 1.0351x over previous
"""Causal self-attention (B=2, T=4096, C=512, H=8, Dh=64) on 8 trn2 cores.

Sharding: core = (batch, head-pair). 2 batches x 4 head-pairs = 8 cores.
Each core computes q/k/v projections for its 2 heads, causal attention in
S^T ([k, q]) layout, and a row-parallel slice of the output projection.
Host sums the 4 partial outputs per batch (+ b_out) and stacks batches.

v2 (from trace analysis of v1: ACT exp is the per-core floor ~160us;
PE was inflated by HAM cold-clock + full-rectangle work; DVE by a 3.3us
[1,512] reciprocal per head-tile):
  - attention operands in bf16 (QT/KT/V/expS/YTn/wo), f32 PSUM accum.
    Whole-pipeline bf16 rounding measured 3.9e-3 rel err in numpy.
  - causal restriction: diagonal-chunk S and AV matmuls only stream the
    visible query range [128r, 512); the stale PSUM/e columns are never
    consumed. Mask multiply shrinks to one [128,128] triangle per
    diagonal chunk (one shared bf16 mask; gpsimd).
  - softmax denominator via the appended ones-column in V (row 64 of
    yt PSUM), reciprocal via the single-instruction
    reciprocal_approx_fast instead of the slow exact reciprocal.
  - exp stays on ACT as [128, 2x512] insts reading 2 PSUM banks.
Device pipeline per core otherwise as v1: per query tile qt, chunk-pair
loop S^T -> exp -> (mask) -> yt += V^T-extended @ expS, with Q/K/V/out
projections software-pipelined into the pair slots so PE never idles
long (HAM stays warm).
"""

import os
import sys

import numpy as np
from ml_dtypes import bfloat16

for _p in ("/opt/trn_rl_repo",):
    if os.path.isdir(_p) and _p not in sys.path:
        sys.path.insert(0, _p)

os.environ.setdefault("MYCRO_LOCAL_CACHE", "1")

import concourse.bass as bass  # noqa: E402
from concourse import bacc  # noqa: E402
import concourse.mybir as mybir  # noqa: E402
import concourse.tile as tile  # noqa: E402
from concourse.bass_utils import run_bass_kernel_spmd  # noqa: E402

F32 = mybir.dt.float32
F32R = mybir.dt.float32r
BF16 = mybir.dt.bfloat16

B, T, C, H, DH = 2, 4096, 512, 8, 64
HEADS_PER_CORE = 2
HD = HEADS_PER_CORE * DH  # 128: head dims owned by one core
N_CORES = 8
QT_TILE = 512  # queries per attention tile
KC = 128  # keys per chunk (contraction granularity)
N_QT = T // QT_TILE  # 8
N_KC = T // KC  # 32
CK = C // 128  # 4 contraction chunks for the projections
SCALE = 1.0 / float(np.sqrt(DH))


def build_program():
    nc = bacc.Bacc(None)

    xT = nc.declare_dram_parameter("xT", [C, T], BF16, isOutput=False)
    wqT = nc.declare_dram_parameter("wqT", [C, HD], BF16, isOutput=False)
    wkT = nc.declare_dram_parameter("wkT", [C, HD], BF16, isOutput=False)
    wvT = nc.declare_dram_parameter("wvT", [C, HD], BF16, isOutput=False)
    # woT[d, h, j]: rows of w_out for this core's head dims, head-split so
    # both heads' slices sit on partitions 0-63. bf16 (matmul operand).
    woT = nc.declare_dram_parameter("woT", [DH, 2, C], BF16, isOutput=False)
    bq = nc.declare_dram_parameter("bq", [HD], F32, isOutput=False)
    bk = nc.declare_dram_parameter("bk", [HD], F32, isOutput=False)
    bv = nc.declare_dram_parameter("bv", [HD], F32, isOutput=False)
    out = nc.declare_dram_parameter("out", [T, C], F32, isOutput=True)

    with tile.TileContext(nc) as tc:
        with (
            tc.tile_pool(name="singles", bufs=1) as singles,
            tc.tile_pool(name="xin", bufs=3) as xin,
            tc.tile_pool(name="exps", bufs=4) as exps,
            tc.tile_pool(name="osb", bufs=3) as osb,
            tc.tile_pool(name="norm", bufs=2) as norm,
            tc.tile_pool(name="ps_proj", bufs=2, space="PSUM") as ps_proj,
            tc.tile_pool(name="ps_s", bufs=2, space="PSUM") as ps_s,
            tc.tile_pool(name="ps_yt", bufs=1, space="PSUM") as ps_yt,
        ):
            # ---- resident inputs -------------------------------------
            # order: wq/wk, then the first x tile (emitted in the qt loop),
            # then everything needed later — keeps the first S chain early
            xT_ap = xT.rearrange("(ko p) t -> p ko t", p=128)
            # tiny bias vectors first (the V-bias broadcast matmul sits early
            # in the PE stream; a late bias DMA head-blocks the whole ramp)
            bq_col = singles.tile([128, 1], F32)
            nc.sync.dma_start(bq_col, bq.rearrange("(p one) -> p one", one=1))
            bk_col = singles.tile([128, 1], F32)
            nc.sync.dma_start(bk_col, bk.rearrange("(p one) -> p one", one=1))
            bv_row = singles.tile([1, HD], F32R)
            nc.sync.dma_start(bv_row, bv[None, :].bitcast(F32R))
            # weights ride the gpsimd SWDGE ring so they overlap the x-tile
            # loads on the sync HWDGE ring at kernel start.
            wqT_sb = singles.tile([128, CK, HD], BF16)
            nc.gpsimd.dma_start(
                wqT_sb, wqT.rearrange("(ko p) m -> p ko m", p=128)
            )
            # first x tile split per contraction chunk so the first q-proj
            # matmul starts after 128KB instead of 512KB of DMA
            xt_first = xin.tile([128, CK, QT_TILE], BF16, tag="xt", name="xt_first")
            for kc in range(CK):
                nc.sync.dma_start(
                    xt_first[:, kc, :],
                    xT_ap[:, kc, bass.ts(0, QT_TILE)],
                )
            wkT_sb = singles.tile([128, CK, HD], BF16)
            nc.gpsimd.dma_start(
                wkT_sb, wkT.rearrange("(ko p) m -> p ko m", p=128)
            )
            wvT_sb = singles.tile([128, CK, HD], BF16)
            nc.gpsimd.dma_start(
                wvT_sb, wvT.rearrange("(ko p) m -> p ko m", p=128)
            )
            woT_sb = singles.tile([DH, 2, C], BF16)
            nc.gpsimd.dma_start(woT_sb, woT[:])



            ones_f32 = singles.tile([128, 128], F32)
            nc.vector.memset(ones_f32, 1.0)
            ones_row = singles.tile([128, 128], F32R)
            nc.vector.tensor_copy(ones_row, ones_f32)

            # one shared [128,128] lower-triangle 0/1 mask (bf16): within a
            # diagonal chunk r the visible block is always q_local >= k_local.
            # Built in F32 (affine_select limitation), then cast.
            tri_f32 = singles.tile([128, 128], F32)
            nc.vector.memset(tri_f32, 1.0)
            nc.gpsimd.affine_select(
                out=tri_f32,
                in_=tri_f32,
                compare_op=mybir.AluOpType.is_ge,
                fill=0.0,
                base=0,
                pattern=[[1, 128]],
                channel_multiplier=-1,
            )
            tri_bf = singles.tile([128, 128], BF16)
            nc.vector.tensor_copy(tri_bf, tri_f32)

            # broadcast bv across partitions via a K=1 matmul
            bias_v_ps = ps_proj.tile([128, HD], F32, tag="psproj")
            nc.tensor.matmul(
                bias_v_ps, ones_row[0:1, :], bv_row, start=True, stop=True
            )
            bias_v_sb = singles.tile([128, HD], F32)
            nc.vector.tensor_copy(bias_v_sb, bias_v_ps)
            bias_v2 = bias_v_sb.rearrange("p (h x) -> p h x", h=2)

            # per-tile storage (separate tile objects -> precise deps)
            QT_t = [
                singles.tile([128, QT_TILE], BF16, name=f"qtt{i}", tag=f"qtt{i}")
                for i in range(N_QT)
            ]
            KT_t = [
                singles.tile([128, QT_TILE], BF16, name=f"ktt{i}", tag=f"ktt{i}")
                for i in range(N_QT)
            ]
            # V chunks in [k, d] layout; per tile: 4 chunks of
            # [V0 | ones | V1 | ones] (65-column stride per head slice)
            V_t = [
                singles.tile([128, 4, 130], BF16, name=f"vt{i}", tag=f"vt{i}")
                for i in range(N_QT)
            ]
            YTn_t = [
                [
                    singles.tile(
                        [64, QT_TILE], BF16, name=f"ytn{h}_{i}", tag=f"ytn{h}_{i}"
                    )
                    for i in range(N_QT)
                ]
                for h in range(2)
            ]
            for i in range(N_QT):
                nc.vector.tensor_copy(V_t[i][:, :, 64:65], ones_f32[:, 0:4, None])
                nc.vector.tensor_copy(
                    V_t[i][:, :, 129:130], ones_f32[:, 0:4, None]
                )

            def emit_qproj(qt, xt):
                ps_q = ps_proj.tile([128, QT_TILE], F32, tag="psproj", name="ps_q")
                for kc in range(CK):
                    nc.tensor.matmul(
                        ps_q,
                        wqT_sb[:, kc, :],
                        xt[:, kc, :],
                        start=(kc == 0),
                        stop=(kc == CK - 1),
                    )
                nc.vector.tensor_scalar_add(QT_t[qt][:], ps_q, bq_col)

            def emit_kproj(qt, xt):
                ps_k = ps_proj.tile([128, QT_TILE], F32, tag="psproj", name="ps_k")
                for kc in range(CK):
                    nc.tensor.matmul(
                        ps_k,
                        wkT_sb[:, kc, :],
                        xt[:, kc, :],
                        start=(kc == 0),
                        stop=(kc == CK - 1),
                    )
                nc.vector.tensor_scalar_add(KT_t[qt][:], ps_k, bk_col)

            def emit_vproj(qt, xt, sv):
                ps_v = ps_proj.tile([128, HD], F32, tag="psproj", name="ps_v")
                for kc in range(CK):
                    nc.tensor.matmul(
                        ps_v,
                        xt[:, kc, bass.ts(sv, 128)],
                        wvT_sb[:, kc, :],
                        start=(kc == 0),
                        stop=(kc == CK - 1),
                    )
                vt = V_t[qt]
                v_vals = bass.AP(
                    tensor=vt.tensor,
                    offset=vt.offset,
                    ap=[vt.ap[0], vt.ap[1], [65, 2], [1, 64]],
                )
                nc.vector.tensor_add(
                    v_vals[:, sv],
                    ps_v[:, 0:HD].rearrange("p (h x) -> p h x", h=2),
                    bias_v2,
                )

            def emit_outproj_sv(qt, sv):
                tc8 = qt * (QT_TILE // 128) + sv
                ps_o = ps_proj.tile([128, C], F32, tag="psproj", name="ps_o")
                for h in range(2):
                    nc.tensor.matmul(
                        ps_o,
                        YTn_t[h][qt][:, bass.ts(sv, 128)],
                        woT_sb[:, h, :],
                        start=(h == 0),
                        stop=(h == 1),
                    )
                o_sb = osb.tile([128, C], F32, tag="osb")
                nc.vector.tensor_copy(o_sb, ps_o)
                nc.sync.dma_start(out[bass.ts(tc8, 128), :], o_sb)

            xt_tiles = {0: xt_first}

            def emit_xt(i):
                if i not in xt_tiles and i < N_QT:
                    xt_i = xin.tile(
                        [128, CK, QT_TILE], BF16, tag="xt", name=f"xt{i}"
                    )
                    nc.sync.dma_start(
                        xt_i, xT_ap[:, :, bass.ts(i, QT_TILE)]
                    )
                    xt_tiles[i] = xt_i

            qproj_done = set()
            pending_norm = []  # (qt, yt_ps) awaiting normalize

            def emit_normalize(nqt, nyt_ps):
                den_sb = norm.tile([128, 2, QT_TILE], F32R, tag="recip")
                bc_sb = [
                    norm.tile([64, QT_TILE], F32, tag=f"bc{h}", name=f"bc{h}")
                    for h in range(2)
                ]
                for h in range(2):
                    nc.vector.tensor_copy(
                        den_sb[64:65, h, :], nyt_ps[h][64:65, :]
                    )
                    bc_ps = ps_proj.tile(
                        [64, QT_TILE], F32, tag="psproj", name="bc_ps"
                    )
                    nc.tensor.matmul(
                        bc_ps,
                        ones_row[64:65, 0:64],
                        den_sb[64:65, h, :],
                        start=True,
                        stop=True,
                    )
                    nc.vector.reciprocal_approx_fast(bc_sb[h], bc_ps)
                    nc.vector.tensor_mul(
                        YTn_t[h][nqt][:], nyt_ps[h][0:64, :], bc_sb[h]
                    )

            for qt in range(N_QT):
                emit_xt(qt)
                xt = xt_tiles[qt]
                if qt not in qproj_done:
                    emit_qproj(qt, xt)
                    qproj_done.add(qt)
                if qt == 0:
                    emit_kproj(qt, xt)
                    for sv in range(4):
                        emit_vproj(qt, xt, sv)

                yt_ps = [
                    ps_yt.tile([128, QT_TILE], F32, tag=f"yt{h}", name=f"yt{h}")
                    for h in range(2)
                ]
                n_pairs = 2 * (qt + 1)
                # spread the pipelined projection work across pair slots so
                # tile boundaries don't burst 30+ matmuls into one slot.
                # deadlines: V chunk sv is consumed at pair 2qt + sv//2;
                # K^T(qt) at pair 2qt (first diagonal pair).
                vproj_at = {}
                outproj_at = {}
                if qt > 0:
                    for sv in range(4):
                        p = min(1 + sv // 2, 2 * qt + sv // 2)
                        vproj_at.setdefault(p, []).append(sv)
                    base = 1 if qt == 1 else 3
                    for sv in range(4):
                        outproj_at.setdefault(
                            min(base + sv, n_pairs - 1), []
                        ).append(sv)
                for pair in range(n_pairs):
                    s_ps = [
                        ps_s.tile(
                            [128, 2, QT_TILE], F32, tag="s", name=f"s{h}"
                        )
                        for h in range(2)
                    ]
                    # sub-major order: the two heads' S matmuls sit on
                    # disjoint PE row-groups (partitions 0-63 / 64-127), so
                    # adjacent emission lets them execute concurrently.
                    for sub in range(2):
                        c = pair * 2 + sub
                        r = c - 4 * qt
                        off = 128 * r if r > 0 else 0
                        for h in range(2):
                            hp = slice(h * 64, h * 64 + 64)
                            nc.tensor.matmul(
                                s_ps[h][:, sub, off:QT_TILE],
                                KT_t[c // 4][hp, bass.ts(c % 4, KC)],
                                QT_t[qt][hp, off:QT_TILE],
                                start=True,
                                stop=True,
                            )
                    # pipelined projections / out-proj for other tiles
                    if pair == 0 and qt > 0:
                        emit_kproj(qt, xt)
                    if pair == 0:
                        while pending_norm:
                            emit_normalize(*pending_norm.pop(0))
                    e_sb = [
                        exps.tile(
                            [128, 2, QT_TILE], BF16, tag=f"e{h}", name=f"e{h}"
                        )
                        for h in range(2)
                    ]
                    for h in range(2):
                        nc.scalar.activation(
                            e_sb[h],
                            s_ps[h],
                            mybir.ActivationFunctionType.Exp,
                            scale=SCALE,
                        )
                        for sub in range(2):
                            c = pair * 2 + sub
                            r = c - 4 * qt
                            if r >= 0:  # diagonal chunk: zero the 128-wide
                                # triangle block (cols < 128r are never read)
                                blk = slice(128 * r, 128 * r + 128)
                                nc.gpsimd.tensor_mul(
                                    e_sb[h][:, sub, blk],
                                    e_sb[h][:, sub, blk],
                                    tri_bf,
                                )
                    for sv in vproj_at.get(pair, []):
                        emit_vproj(qt, xt, sv)
                    for h in range(2):
                        for sub in range(2):
                            c = pair * 2 + sub
                            r = c - 4 * qt
                            off = 128 * r if r > 0 else 0
                            nc.tensor.matmul(
                                yt_ps[h][0:65, off:QT_TILE],
                                V_t[c // 4][:, c % 4, h * 65 : h * 65 + 65],
                                e_sb[h][:, sub, off:QT_TILE],
                                start=(pair == 0 and sub == 0),
                                stop=(pair == n_pairs - 1 and sub == 1),
                            )
                    for sv in outproj_at.get(pair, []):
                        emit_outproj_sv(qt - 1, sv)
                    if pair == min(2, n_pairs - 1) and qt + 1 < N_QT:
                        emit_xt(qt + 1)
                        emit_qproj(qt + 1, xt_tiles[qt + 1])
                        qproj_done.add(qt + 1)

                # normalize (row 64 = softmax denominator) is deferred
                # into the next tile's pair-0 slot so the den-copy -> bc
                # matmul chain overlaps S/exp instead of stalling PE at the
                # tile boundary.
                pending_norm.append((qt, yt_ps))
            while pending_norm:
                emit_normalize(*pending_norm.pop(0))
            for sv in range(4):
                emit_outproj_sv(N_QT - 1, sv)

    return nc


_PROGRAM = None


def _get_program():
    global _PROGRAM
    if _PROGRAM is None:
        _PROGRAM = build_program()
        if not _PROGRAM.is_finalized():
            _PROGRAM.finalize()
    return _PROGRAM


def make_in_maps(x, w_qkv, b_qkv, w_out, b_out):
    """Shard the full inputs into per-core input maps."""
    x = np.ascontiguousarray(x, dtype=np.float32)
    w_qkv = np.ascontiguousarray(w_qkv, dtype=np.float32)
    b_qkv = np.ascontiguousarray(b_qkv, dtype=np.float32)
    w_out = np.ascontiguousarray(w_out, dtype=np.float32)

    wq = w_qkv[0:C]  # [C, C] rows = q features
    wk = w_qkv[C : 2 * C]
    wv = w_qkv[2 * C : 3 * C]
    bq_full = b_qkv[0:C]
    bk_full = b_qkv[C : 2 * C]
    bv_full = b_qkv[2 * C : 3 * C]

    xT_b = [np.ascontiguousarray(x[b].T.astype(bfloat16)) for b in range(B)]

    in_maps = []
    for core in range(N_CORES):
        b = core // 4
        g = core % 4
        rows = slice(g * HD, (g + 1) * HD)  # this core's head dims
        woT = np.ascontiguousarray(
            w_out[:, rows].T.reshape(2, DH, C).transpose(1, 0, 2)
        ).astype(bfloat16)  # [DH, 2, C]
        in_maps.append(
            {
                "xT": xT_b[b],
                "wqT": np.ascontiguousarray(wq[rows].T.astype(bfloat16)),
                "wkT": np.ascontiguousarray(wk[rows].T.astype(bfloat16)),
                "wvT": np.ascontiguousarray(wv[rows].T.astype(bfloat16)),
                "woT": woT,
                "bq": np.ascontiguousarray(bq_full[rows]),
                "bk": np.ascontiguousarray(bk_full[rows]),
                "bv": np.ascontiguousarray(bv_full[rows]),
            }
        )
    return in_maps


def kernel(x, w_qkv, b_qkv, w_out, b_out, _trace=False, _trace_kwargs=None):
    in_maps = make_in_maps(x, w_qkv, b_qkv, w_out, b_out)
    nc = _get_program()
    res = run_bass_kernel_spmd(
        nc,
        in_maps,
        list(range(N_CORES)),
        trace=_trace,
        **(_trace_kwargs or {}),
    )
    outs = [res.results[c]["out"] for c in range(N_CORES)]
    bo = np.asarray(b_out, dtype=np.float32)
    # unshard: sum the 4 row-parallel partials per batch (+ bias), stack
    y = np.stack(
        [
            outs[0] + outs[1] + outs[2] + outs[3] + bo,
            outs[4] + outs[5] + outs[6] + outs[7] + bo,
        ]
    ).astype(np.float32)
    if _trace:
        return y, res
    return y


# revision 13
# speedup vs baseline: 1.1203x; 1.0545x over previous
"""Causal self-attention (B=2, T=4096, C=512, H=8, Dh=64) on 8 trn2 cores.

Sharding: core = (batch, head-pair). 2 batches x 4 head-pairs = 8 cores.
Each core computes q/k/v projections for its 2 heads, causal attention in
S^T ([k, q]) layout, and a row-parallel slice of the output projection.
Host sums the 4 partial outputs per batch (+ b_out) and stacks batches.

v2 (from trace analysis of v1: ACT exp is the per-core floor ~160us;
PE was inflated by HAM cold-clock + full-rectangle work; DVE by a 3.3us
[1,512] reciprocal per head-tile):
  - attention operands in bf16 (QT/KT/V/expS/YTn/wo), f32 PSUM accum.
    Whole-pipeline bf16 rounding measured 3.9e-3 rel err in numpy.
  - causal restriction: diagonal-chunk S and AV matmuls only stream the
    visible query range [128r, 512); the stale PSUM/e columns are never
    consumed. Mask multiply shrinks to one [128,128] triangle per
    diagonal chunk (one shared bf16 mask; gpsimd).
  - softmax denominator via the appended ones-column in V (row 64 of
    yt PSUM), reciprocal via the single-instruction
    reciprocal_approx_fast instead of the slow exact reciprocal.
  - exp stays on ACT as [128, 2x512] insts reading 2 PSUM banks.
Device pipeline per core otherwise as v1: per query tile qt, chunk-pair
loop S^T -> exp -> (mask) -> yt += V^T-extended @ expS, with Q/K/V/out
projections software-pipelined into the pair slots so PE never idles
long (HAM stays warm).
"""

import os
import sys

import numpy as np
from ml_dtypes import bfloat16

for _p in ("/opt/trn_rl_repo",):
    if os.path.isdir(_p) and _p not in sys.path:
        sys.path.insert(0, _p)

os.environ.setdefault("MYCRO_LOCAL_CACHE", "1")

import concourse.bass as bass  # noqa: E402
from concourse import bacc  # noqa: E402
import concourse.mybir as mybir  # noqa: E402
import concourse.tile as tile  # noqa: E402
from concourse.bass_utils import run_bass_kernel_spmd  # noqa: E402

F32 = mybir.dt.float32
F32R = mybir.dt.float32r
BF16 = mybir.dt.bfloat16

B, T, C, H, DH = 2, 4096, 512, 8, 64
HEADS_PER_CORE = 2
HD = HEADS_PER_CORE * DH  # 128: head dims owned by one core
N_CORES = 8
QT_TILE = 512  # queries per attention tile
KC = 128  # keys per chunk (contraction granularity)
N_QT = T // QT_TILE  # 8
N_KC = T // KC  # 32
CK = C // 128  # 4 contraction chunks for the projections
SCALE = 1.0 / float(np.sqrt(DH))


def build_program():
    nc = bacc.Bacc(None)

    xT = nc.declare_dram_parameter("xT", [C, T], BF16, isOutput=False)
    wqT = nc.declare_dram_parameter("wqT", [C, HD], BF16, isOutput=False)
    wkT = nc.declare_dram_parameter("wkT", [C, HD], BF16, isOutput=False)
    wvT = nc.declare_dram_parameter("wvT", [C, HD], BF16, isOutput=False)
    # woT[d, h, j]: rows of w_out for this core's head dims, head-split so
    # both heads' slices sit on partitions 0-63. bf16 (matmul operand).
    woT = nc.declare_dram_parameter("woT", [DH, 2, C], BF16, isOutput=False)
    bq = nc.declare_dram_parameter("bq", [HD], F32, isOutput=False)
    bk = nc.declare_dram_parameter("bk", [HD], F32, isOutput=False)
    bv = nc.declare_dram_parameter("bv", [HD], F32, isOutput=False)
    out = nc.declare_dram_parameter("out", [T, C], F32, isOutput=True)

    with tile.TileContext(nc) as tc:
        with (
            tc.tile_pool(name="singles", bufs=1) as singles,
            tc.tile_pool(name="xin", bufs=3) as xin,
            tc.tile_pool(name="exps", bufs=4) as exps,
            tc.tile_pool(name="osb", bufs=3) as osb,
            tc.tile_pool(name="norm", bufs=2) as norm,
            tc.tile_pool(name="ps_s", bufs=3, space="PSUM") as ps_s,
            tc.tile_pool(name="ps_yt", bufs=1, space="PSUM") as ps_yt,
        ):
            # ---- resident inputs -------------------------------------
            # order: wq/wk, then the first x tile (emitted in the qt loop),
            # then everything needed later — keeps the first S chain early
            xT_ap = xT.rearrange("(ko p) t -> p ko t", p=128)
            # tiny bias vectors first (the V-bias broadcast matmul sits early
            # in the PE stream; a late bias DMA head-blocks the whole ramp)
            bq_col = singles.tile([128, 1], F32)
            nc.sync.dma_start(bq_col, bq.rearrange("(p one) -> p one", one=1))
            bk_col = singles.tile([128, 1], F32)
            nc.sync.dma_start(bk_col, bk.rearrange("(p one) -> p one", one=1))
            bv_row = singles.tile([1, HD], F32R)
            nc.sync.dma_start(bv_row, bv[None, :].bitcast(F32R))
            # weights ride the gpsimd SWDGE ring so they overlap the x-tile
            # loads on the sync HWDGE ring at kernel start.
            wqT_sb = singles.tile([128, CK, HD], BF16)
            nc.gpsimd.dma_start(
                wqT_sb, wqT.rearrange("(ko p) m -> p ko m", p=128)
            )
            # first x tile split per contraction chunk so the first q-proj
            # matmul starts after 128KB instead of 512KB of DMA
            xt_first = xin.tile([128, CK, QT_TILE], BF16, tag="xt", name="xt_first")
            for kc in range(CK):
                nc.sync.dma_start(
                    xt_first[:, kc, :],
                    xT_ap[:, kc, bass.ts(0, QT_TILE)],
                )
            wkT_sb = singles.tile([128, CK, HD], BF16)
            nc.gpsimd.dma_start(
                wkT_sb, wkT.rearrange("(ko p) m -> p ko m", p=128)
            )
            wvT_sb = singles.tile([128, CK, HD], BF16)
            nc.gpsimd.dma_start(
                wvT_sb, wvT.rearrange("(ko p) m -> p ko m", p=128)
            )
            woT_sb = singles.tile([DH, 2, C], BF16)
            nc.gpsimd.dma_start(woT_sb, woT[:])



            ones_f32 = singles.tile([128, 128], F32)
            nc.vector.memset(ones_f32, 1.0)
            ones_row = singles.tile([128, 128], F32R)
            nc.vector.tensor_copy(ones_row, ones_f32)

            # one shared [128,128] lower-triangle 0/1 mask (bf16): within a
            # diagonal chunk r the visible block is always q_local >= k_local.
            # Built in F32 (affine_select limitation), then cast.
            tri_f32 = singles.tile([128, 128], F32)
            nc.vector.memset(tri_f32, 1.0)
            nc.gpsimd.affine_select(
                out=tri_f32,
                in_=tri_f32,
                compare_op=mybir.AluOpType.is_ge,
                fill=0.0,
                base=0,
                pattern=[[1, 128]],
                channel_multiplier=-1,
            )
            tri_bf = singles.tile([128, 128], BF16)
            nc.vector.tensor_copy(tri_bf, tri_f32)

            # broadcast bv across partitions via a K=1 matmul
            bias_v_ps = ps_s.tile([128, HD], F32, tag="s", name="bias_v_ps")
            nc.tensor.matmul(
                bias_v_ps, ones_row[0:1, :], bv_row, start=True, stop=True
            )
            bias_v_sb = singles.tile([128, HD], F32)
            nc.vector.tensor_copy(bias_v_sb, bias_v_ps)
            bias_v2 = bias_v_sb.rearrange("p (h x) -> p h x", h=2)

            # per-tile storage (separate tile objects -> precise deps)
            QT_t = [
                singles.tile([128, QT_TILE], BF16, name=f"qtt{i}", tag=f"qtt{i}")
                for i in range(N_QT)
            ]
            KT_t = [
                singles.tile([128, QT_TILE], BF16, name=f"ktt{i}", tag=f"ktt{i}")
                for i in range(N_QT)
            ]
            # V chunks in [k, d] layout; per tile: 4 chunks of
            # [V0 | ones | V1 | ones] (65-column stride per head slice)
            V_t = [
                singles.tile([128, 4, 130], BF16, name=f"vt{i}", tag=f"vt{i}")
                for i in range(N_QT)
            ]
            YTn_t = [
                [
                    singles.tile(
                        [64, QT_TILE], BF16, name=f"ytn{h}_{i}", tag=f"ytn{h}_{i}"
                    )
                    for i in range(N_QT)
                ]
                for h in range(2)
            ]
            for i in range(N_QT):
                nc.vector.tensor_copy(V_t[i][:, :, 64:65], ones_f32[:, 0:4, None])
                nc.vector.tensor_copy(
                    V_t[i][:, :, 129:130], ones_f32[:, 0:4, None]
                )

            def emit_qproj(qt, xt):
                ps_q = ps_s.tile([128, QT_TILE], F32, tag="s", name="ps_q")
                for kc in range(CK):
                    nc.tensor.matmul(
                        ps_q,
                        wqT_sb[:, kc, :],
                        xt[:, kc, :],
                        start=(kc == 0),
                        stop=(kc == CK - 1),
                    )
                nc.vector.tensor_scalar_add(QT_t[qt][:], ps_q, bq_col)

            def emit_kproj(qt, xt):
                ps_k = ps_s.tile([128, QT_TILE], F32, tag="s", name="ps_k")
                for kc in range(CK):
                    nc.tensor.matmul(
                        ps_k,
                        wkT_sb[:, kc, :],
                        xt[:, kc, :],
                        start=(kc == 0),
                        stop=(kc == CK - 1),
                    )
                nc.vector.tensor_scalar_add(KT_t[qt][:], ps_k, bk_col)

            def emit_vproj(qt, xt, sv):
                ps_v = ps_s.tile([128, HD], F32, tag="s", name="ps_v")
                for kc in range(CK):
                    nc.tensor.matmul(
                        ps_v,
                        xt[:, kc, bass.ts(sv, 128)],
                        wvT_sb[:, kc, :],
                        start=(kc == 0),
                        stop=(kc == CK - 1),
                    )
                vt = V_t[qt]
                v_vals = bass.AP(
                    tensor=vt.tensor,
                    offset=vt.offset,
                    ap=[vt.ap[0], vt.ap[1], [65, 2], [1, 64]],
                )
                nc.vector.tensor_add(
                    v_vals[:, sv],
                    ps_v[:, 0:HD].rearrange("p (h x) -> p h x", h=2),
                    bias_v2,
                )

            def emit_outproj_sv(qt, sv):
                tc8 = qt * (QT_TILE // 128) + sv
                ps_o = ps_s.tile([128, C], F32, tag="s", name="ps_o")
                for h in range(2):
                    nc.tensor.matmul(
                        ps_o,
                        YTn_t[h][qt][:, bass.ts(sv, 128)],
                        woT_sb[:, h, :],
                        start=(h == 0),
                        stop=(h == 1),
                    )
                o_sb = osb.tile([128, C], F32, tag="osb")
                nc.vector.tensor_copy(o_sb, ps_o)
                nc.sync.dma_start(out[bass.ts(tc8, 128), :], o_sb)

            xt_tiles = {0: xt_first}

            def emit_xt(i):
                if i not in xt_tiles and i < N_QT:
                    xt_i = xin.tile(
                        [128, CK, QT_TILE], BF16, tag="xt", name=f"xt{i}"
                    )
                    nc.sync.dma_start(
                        xt_i, xT_ap[:, :, bass.ts(i, QT_TILE)]
                    )
                    xt_tiles[i] = xt_i

            qproj_done = set()
            pending_norm = []  # (qt, yt_ps) awaiting normalize

            def emit_normalize(nqt, nyt_ps):
                den_sb = norm.tile([128, 2, QT_TILE], F32R, tag="recip")
                bc_sb = [
                    norm.tile([64, QT_TILE], F32, tag=f"bc{h}", name=f"bc{h}")
                    for h in range(2)
                ]
                for h in range(2):
                    nc.vector.tensor_copy(
                        den_sb[64:65, h, :], nyt_ps[h][64:65, :]
                    )
                    bc_ps = ps_s.tile(
                        [64, QT_TILE], F32, tag="s", name="bc_ps"
                    )
                    nc.tensor.matmul(
                        bc_ps,
                        ones_row[64:65, 0:64],
                        den_sb[64:65, h, :],
                        start=True,
                        stop=True,
                    )
                    nc.vector.reciprocal_approx_fast(bc_sb[h], bc_ps)
                    nc.vector.tensor_mul(
                        YTn_t[h][nqt][:], nyt_ps[h][0:64, :], bc_sb[h]
                    )

            for qt in range(N_QT):
                emit_xt(qt)
                xt = xt_tiles[qt]
                if qt not in qproj_done:
                    emit_qproj(qt, xt)
                    qproj_done.add(qt)
                if qt == 0:
                    emit_kproj(qt, xt)
                    for sv in range(4):
                        emit_vproj(qt, xt, sv)

                yt_ps = [
                    ps_yt.tile([128, QT_TILE], F32, tag=f"yt{h}", name=f"yt{h}")
                    for h in range(2)
                ]
                n_pairs = 2 * (qt + 1)
                # spread the pipelined projection work across pair slots so
                # tile boundaries don't burst 30+ matmuls into one slot.
                # deadlines: V chunk sv is consumed at pair 2qt + sv//2;
                # K^T(qt) at pair 2qt (first diagonal pair).
                vproj_at = {}
                outproj_at = {}
                if qt > 0:
                    for sv in range(4):
                        p = min(1 + sv // 2, 2 * qt + sv // 2)
                        vproj_at.setdefault(p, []).append(sv)
                    base = 1 if qt == 1 else 3
                    for sv in range(4):
                        outproj_at.setdefault(
                            min(base + sv, n_pairs - 1), []
                        ).append(sv)
                for pair in range(n_pairs):
                    s_ps = [
                        ps_s.tile(
                            [128, 2, QT_TILE], F32, tag="s", name=f"s{h}"
                        )
                        for h in range(2)
                    ]
                    # sub-major order: the two heads' S matmuls sit on
                    # disjoint PE row-groups (partitions 0-63 / 64-127), so
                    # adjacent emission lets them execute concurrently.
                    for sub in range(2):
                        c = pair * 2 + sub
                        r = c - 4 * qt
                        off = 128 * r if r > 0 else 0
                        for h in range(2):
                            hp = slice(h * 64, h * 64 + 64)
                            nc.tensor.matmul(
                                s_ps[h][:, sub, off:QT_TILE],
                                KT_t[c // 4][hp, bass.ts(c % 4, KC)],
                                QT_t[qt][hp, off:QT_TILE],
                                start=True,
                                stop=True,
                            )
                    # pipelined projections / out-proj for other tiles
                    if pair == 0 and qt > 0:
                        emit_kproj(qt, xt)
                    if pair == 0:
                        while pending_norm:
                            emit_normalize(*pending_norm.pop(0))
                    e_sb = [
                        exps.tile(
                            [128, 2, QT_TILE], BF16, tag=f"e{h}", name=f"e{h}"
                        )
                        for h in range(2)
                    ]
                    for h in range(2):
                        nc.scalar.activation(
                            e_sb[h],
                            s_ps[h],
                            mybir.ActivationFunctionType.Exp,
                            scale=SCALE,
                        )
                        for sub in range(2):
                            c = pair * 2 + sub
                            r = c - 4 * qt
                            if r >= 0:  # diagonal chunk: zero the 128-wide
                                # triangle block (cols < 128r are never read)
                                blk = slice(128 * r, 128 * r + 128)
                                nc.gpsimd.tensor_mul(
                                    e_sb[h][:, sub, blk],
                                    e_sb[h][:, sub, blk],
                                    tri_bf,
                                )
                    for sv in vproj_at.get(pair, []):
                        emit_vproj(qt, xt, sv)
                    for h in range(2):
                        for sub in range(2):
                            c = pair * 2 + sub
                            r = c - 4 * qt
                            off = 128 * r if r > 0 else 0
                            nc.tensor.matmul(
                                yt_ps[h][0:65, off:QT_TILE],
                                V_t[c // 4][:, c % 4, h * 65 : h * 65 + 65],
                                e_sb[h][:, sub, off:QT_TILE],
                                start=(pair == 0 and sub == 0),
                                stop=(pair == n_pairs - 1 and sub == 1),
                            )
                    for sv in outproj_at.get(pair, []):
                        emit_outproj_sv(qt - 1, sv)
                    if pair == min(2, n_pairs - 1) and qt + 1 < N_QT:
                        emit_xt(qt + 1)
                        emit_qproj(qt + 1, xt_tiles[qt + 1])
                        qproj_done.add(qt + 1)

                # normalize (row 64 = softmax denominator) is deferred
                # into the next tile's pair-0 slot so the den-copy -> bc
                # matmul chain overlaps S/exp instead of stalling PE at the
                # tile boundary.
                pending_norm.append((qt, yt_ps))
            while pending_norm:
                emit_normalize(*pending_norm.pop(0))
            for sv in range(4):
                emit_outproj_sv(N_QT - 1, sv)

    return nc


_PROGRAM = None


def _get_program():
    global _PROGRAM
    if _PROGRAM is None:
        _PROGRAM = build_program()
        if not _PROGRAM.is_finalized():
            _PROGRAM.finalize()
    return _PROGRAM


def make_in_maps(x, w_qkv, b_qkv, w_out, b_out):
    """Shard the full inputs into per-core input maps."""
    x = np.ascontiguousarray(x, dtype=np.float32)
    w_qkv = np.ascontiguousarray(w_qkv, dtype=np.float32)
    b_qkv = np.ascontiguousarray(b_qkv, dtype=np.float32)
    w_out = np.ascontiguousarray(w_out, dtype=np.float32)

    wq = w_qkv[0:C]  # [C, C] rows = q features
    wk = w_qkv[C : 2 * C]
    wv = w_qkv[2 * C : 3 * C]
    bq_full = b_qkv[0:C]
    bk_full = b_qkv[C : 2 * C]
    bv_full = b_qkv[2 * C : 3 * C]

    xT_b = [np.ascontiguousarray(x[b].T.astype(bfloat16)) for b in range(B)]

    in_maps = []
    for core in range(N_CORES):
        b = core // 4
        g = core % 4
        rows = slice(g * HD, (g + 1) * HD)  # this core's head dims
        woT = np.ascontiguousarray(
            w_out[:, rows].T.reshape(2, DH, C).transpose(1, 0, 2)
        ).astype(bfloat16)  # [DH, 2, C]
        in_maps.append(
            {
                "xT": xT_b[b],
                "wqT": np.ascontiguousarray(wq[rows].T.astype(bfloat16)),
                "wkT": np.ascontiguousarray(wk[rows].T.astype(bfloat16)),
                "wvT": np.ascontiguousarray(wv[rows].T.astype(bfloat16)),
                "woT": woT,
                "bq": np.ascontiguousarray(bq_full[rows]),
                "bk": np.ascontiguousarray(bk_full[rows]),
                "bv": np.ascontiguousarray(bv_full[rows]),
            }
        )
    return in_maps


def kernel(x, w_qkv, b_qkv, w_out, b_out, _trace=False, _trace_kwargs=None):
    in_maps = make_in_maps(x, w_qkv, b_qkv, w_out, b_out)
    nc = _get_program()
    res = run_bass_kernel_spmd(
        nc,
        in_maps,
        list(range(N_CORES)),
        trace=_trace,
        **(_trace_kwargs or {}),
    )
    outs = [res.results[c]["out"] for c in range(N_CORES)]
    bo = np.asarray(b_out, dtype=np.float32)
    # unshard: sum the 4 row-parallel partials per batch (+ bias), stack
    y = np.stack(
        [
            outs[0] + outs[1] + outs[2] + outs[3] + bo,
            outs[4] + outs[5] + outs[6] + outs[7] + bo,
        ]
    ).astype(np.float32)
    if _trace:
        return y, res
    return y
